# revision 6
# baseline (speedup 1.0000x reference)
"""WaveRNN autoregressive sampling kernel for Trainium2 (8 NeuronCores).

Strategy: data-parallel over batch (B=16 -> 2 sequences per core). All
weights + the whole per-core cond/gumbel streams live in SBUF; the 2000-step
scan runs fully unrolled on-device with feature-major (weight-stationary)
fp32 matmuls. Sampling reproduces jax.random.categorical exactly via
precomputed Gumbel noise + argmax (computed as is_equal one-hot, which also
feeds the next step's embedding lookup as a one-hot matmul).

GRU math decomposition (x = cond_t + emb[prev]):
  gi = x@W_ih.T + b_ih = cond_t@W_ih.T + E2[prev] + b_ih   with E2 = emb@W_ih.T
  r,z need only gi+gh sums; n needs i_n and h_n separately, so the PSUM tile
  keeps three regions: rz-sum (cols 0:16), h_n (cols 16:24), i_n (cols 24:32).
sigmoid(x) computed as 0.5*tanh(0.5x)+0.5 (XLA's logistic expansion; ACT tanh
is the 4-ULP table vs sigmoid's 40-ULP).
"""

import numpy as np

# problem constants (nn_ExportableWaveRNN: B,T,C,G,H,V)
B, T, C, G, H, V = 16, 2000, 128, 512, 512, 256
NCORES = 8
BPC = B // NCORES  # 2
OC3 = 12  # 3G/128 output chunks of the gate matmul
GC = 4    # G/128
HC = 4    # H/128
VC = 2    # V/128
TPAD = 2048  # padded T for the cond transpose tiles (32 tiles of 128 rows)

_CACHE = {}


def _gumbel_noise():
    """[T, B, V] f32 gumbel noise exactly as jax.random.categorical draws it."""
    if "gum" in _CACHE:
        return _CACHE["gum"]
    import jax
    import jax.numpy as jnp

    cpu = jax.devices("cpu")[0]
    with jax.default_device(cpu):
        keys = jax.random.split(jax.random.key(1), T)
        g = jax.lax.map(lambda k: jax.random.gumbel(k, (B, V), jnp.float32), keys)
        g = np.asarray(g)
    _CACHE["gum"] = g
    return g


def _build_program(t_steps):
    key = ("prog", t_steps)
    if key in _CACHE:
        return _CACHE[key]
    from concourse import bacc
    import concourse.mybir as mybir
    from concourse.tile import TileContext
    from concourse.bass import ds

    f32 = mybir.dt.float32
    AF = mybir.ActivationFunctionType
    ALU = mybir.AluOpType
    AX = mybir.AxisListType

    nc = bacc.Bacc()

    def din(name, shape):
        return nc.dram_tensor(name, shape, f32, kind="ExternalInput")

    wih = din("wih", [128, OC3 * 128])
    wehh = din("wehh", [128, GC * OC3 * 128])
    e2m = din("e2m", [128, VC * OC3 * 128])
    whid = din("whid", [128, GC * HC * 128])
    wout = din("wout", [128, HC * VC * 128])
    ident = din("ident", [128, 128])
    iota = din("iota", [128, VC])
    brz = din("brz", [128, 16])
    bin_ = din("bin", [128, 8])
    bhhn = din("bhhn", [128, 8])
    bhid = din("bhid", [128, 8])
    bout = din("bout", [128, 4])
    cond_n = din("cond_n", [TPAD * BPC, 128])  # rows r = t*BPC+b
    gum = din("gum", [128, t_steps * 4])
    h0f = din("h0f", [128, 2 * GC])
    oneh0 = din("oneh0", [128, 2 * VC])

    out_logits = nc.dram_tensor("out_logits", [128, t_steps * 4], f32, kind="ExternalOutput")
    out_samples = nc.dram_tensor("out_samples", [2, t_steps], f32, kind="ExternalOutput")
    out_h = nc.dram_tensor("out_h", [128, 2 * GC], f32, kind="ExternalOutput")

    with TileContext(nc) as tc:
        with (
            tc.tile_pool(name="cpool", bufs=1) as cpool,
            tc.tile_pool(name="stg", bufs=3) as stg,
            tc.tile_pool(name="work", bufs=3) as work,
            tc.tile_pool(name="psg", bufs=2, space="PSUM") as psg,
            tc.tile_pool(name="psh", bufs=1, space="PSUM") as psh,
            tc.tile_pool(name="psl", bufs=2, space="PSUM") as psl,
            tc.tile_pool(name="pst", bufs=1, space="PSUM") as pst,
            tc.tile_pool(name="pso", bufs=1, space="PSUM") as pso,
            tc.tile_pool(name="psi", bufs=1, space="PSUM") as psi,
        ):
            # ---- resident constants ----
            t_wih = cpool.tile([128, OC3 * 128], f32)
            t_wehh = cpool.tile([128, GC * OC3 * 128], f32)
            t_e2 = cpool.tile([128, VC * OC3 * 128], f32)
            t_whid = cpool.tile([128, GC * HC * 128], f32)
            t_wout = cpool.tile([128, HC * VC * 128], f32)
            t_ident = cpool.tile([128, 128], f32)
            t_iota = cpool.tile([128, VC], f32)
            t_brz = cpool.tile([128, 16], f32)
            t_bin = cpool.tile([128, 8], f32)
            t_bhhn = cpool.tile([128, 8], f32)
            t_bhid = cpool.tile([128, 8], f32)
            t_bout = cpool.tile([128, 4], f32)
            t_gum = cpool.tile([128, t_steps * 4], f32)
            for tt, src in [
                (t_wih, wih), (t_wehh, wehh), (t_e2, e2m), (t_whid, whid),
                (t_wout, wout), (t_ident, ident), (t_iota, iota), (t_brz, brz),
                (t_bin, bin_), (t_bhhn, bhhn), (t_bhid, bhid), (t_bout, bout),
                (t_gum, gum),
            ]:
                nc.sync.dma_start(tt[:], src[:])

            # ---- state ----
            t_h = cpool.tile([128, 2 * GC], f32)
            t_oneh = cpool.tile([128, 2 * VC], f32)
            nc.sync.dma_start(t_h[:], h0f[:])
            nc.sync.dma_start(t_oneh[:], oneh0[:])

            # ---- transpose cond into feature-major condT [c, t*2+b] ----
            t_condT = cpool.tile([128, TPAD * BPC], f32)
            n_ct = (t_steps * BPC + 127) // 128
            for rt in range(n_ct):
                cstg = stg.tile([128, 128], f32, name=f"cstg{rt}", tag="cstg")
                nc.sync.dma_start(cstg[:], cond_n[rt * 128:(rt + 1) * 128, :])
                pct = psg.tile([128, 128], f32, name=f"pct{rt}", tag="pg")
                nc.tensor.transpose(pct[:], cstg[:], t_ident[:])
                nc.vector.tensor_copy(t_condT[:, rt * 128:(rt + 1) * 128], pct[:])

            # ---- the scan: For_i over bodies of U unrolled steps ----
            U = 50 if t_steps % 50 == 0 else t_steps
            n_iter = t_steps // U
            condw = cpool.tile([128, U * 2], f32)
            gumw = cpool.tile([128, U * 4], f32)
            logitsw = cpool.tile([128, U * 4], f32)
            samplesw = cpool.tile([2, U], f32)

            def body(i):
                nc.vector.tensor_copy(condw[:], t_condT[:, ds(i * (U * 2), U * 2)])
                nc.vector.tensor_copy(gumw[:], t_gum[:, ds(i * (U * 4), U * 4)])
                for t in range(U):
                    pg = psg.tile([128, 32], f32, name=f"pg{t}", tag="pg")
                    first = True

                    def gcol(oc, hh):
                        if oc < 8:
                            return oc * 2
                        return (16 if hh else 24) + (oc - 8) * 2

                    for oc in range(OC3):
                        j = gcol(oc, hh=False)
                        nc.tensor.matmul(
                            pg[:, j:j + 2],
                            t_wih[:, oc * 128:(oc + 1) * 128],
                            condw[:, t * 2:t * 2 + 2],
                            start=first, stop=False,
                        )
                        first = False
                    for kc in range(GC):
                        for oc in range(OC3):
                            j = gcol(oc, hh=True)
                            nc.tensor.matmul(
                                pg[:, j:j + 2],
                                t_wehh[:, (kc * OC3 + oc) * 128:(kc * OC3 + oc + 1) * 128],
                                t_h[:, kc * 2:kc * 2 + 2],
                                start=False, stop=False,
                            )
                    for vc in range(VC):
                        for oc in range(OC3):
                            j = gcol(oc, hh=False)
                            last = (vc == VC - 1) and (oc == OC3 - 1)
                            nc.tensor.matmul(
                                pg[:, j:j + 2],
                                t_e2[:, (vc * OC3 + oc) * 128:(vc * OC3 + oc + 1) * 128],
                                t_oneh[:, vc * 2:vc * 2 + 2],
                                start=False, stop=last,
                            )

                    rzs = work.tile([128, 16], f32, name=f"rzs{t}", tag="rzs")
                    nc.vector.tensor_add(rzs[:], pg[:, 0:16], t_brz[:])
                    rzt = work.tile([128, 16], f32, name=f"rzt{t}", tag="rzt")
                    nc.scalar.activation(rzt[:], rzs[:], AF.Tanh, scale=0.5)
                    rz = work.tile([128, 16], f32, name=f"rz{t}", tag="rz")
                    nc.vector.tensor_scalar(rz[:], rzt[:], 0.5, 0.5, ALU.mult, ALU.add)
                    hn = work.tile([128, 8], f32, name=f"hn{t}", tag="hn")
                    nc.vector.tensor_add(hn[:], pg[:, 16:24], t_bhhn[:])
                    inp = work.tile([128, 8], f32, name=f"inp{t}", tag="inp")
                    nc.vector.tensor_add(inp[:], pg[:, 24:32], t_bin[:])
                    rhn = work.tile([128, 8], f32, name=f"rhn{t}", tag="rhn")
                    nc.vector.tensor_mul(rhn[:], rz[:, 0:8], hn[:])
                    npre = work.tile([128, 8], f32, name=f"npre{t}", tag="npre")
                    nc.vector.tensor_add(npre[:], inp[:], rhn[:])
                    nn_ = work.tile([128, 8], f32, name=f"nn{t}", tag="nn")
                    nc.scalar.activation(nn_[:], npre[:], AF.Tanh)
                    omz = work.tile([128, 8], f32, name=f"omz{t}", tag="omz")
                    nc.vector.tensor_scalar(omz[:], rz[:, 8:16], -1.0, 1.0, ALU.mult, ALU.add)
                    t2 = work.tile([128, 8], f32, name=f"t2_{t}", tag="t2")
                    nc.vector.tensor_mul(t2[:], omz[:], nn_[:])
                    t3 = work.tile([128, 8], f32, name=f"t3_{t}", tag="t3")
                    nc.vector.tensor_mul(t3[:], rz[:, 8:16], t_h[:])
                    nc.vector.tensor_add(t_h[:], t2[:], t3[:])

                    ph = psh.tile([128, 8], f32, name=f"ph{t}", tag="ph")
                    for kc in range(GC):
                        for hc in range(HC):
                            nc.tensor.matmul(
                                ph[:, hc * 2:hc * 2 + 2],
                                t_whid[:, (kc * HC + hc) * 128:(kc * HC + hc + 1) * 128],
                                t_h[:, kc * 2:kc * 2 + 2],
                                start=(kc == 0 and hc == 0), stop=(kc == GC - 1 and hc == HC - 1),
                            )
                    hpre = work.tile([128, 8], f32, name=f"hpre{t}", tag="hpre")
                    nc.vector.tensor_add(hpre[:], ph[:], t_bhid[:])
                    hid = work.tile([128, 8], f32, name=f"hid{t}", tag="hid")
                    nc.vector.tensor_scalar_max(hid[:], hpre[:], 0.0)

                    pl = psl.tile([128, 4], f32, name=f"pl{t}", tag="pl")
                    for kc in range(HC):
                        for vc in range(VC):
                            nc.tensor.matmul(
                                pl[:, vc * 2:vc * 2 + 2],
                                t_wout[:, (kc * VC + vc) * 128:(kc * VC + vc + 1) * 128],
                                hid[:, kc * 2:kc * 2 + 2],
                                start=(kc == 0 and vc == 0), stop=(kc == HC - 1 and vc == VC - 1),
                            )
                    lt = logitsw[:, t * 4:(t + 1) * 4]
                    nc.vector.tensor_add(lt, pl[:], t_bout[:])
                    y = work.tile([128, 4], f32, name=f"y{t}", tag="y")
                    nc.vector.tensor_add(y[:], lt, gumw[:, t * 4:(t + 1) * 4])

                    ptb = pst.tile([2, 256], f32, name=f"ptb{t}", tag="pt")
                    nc.tensor.matmul(ptb[:, 0:128], y[:, 0:2], t_ident[:],
                                     is_transpose=True, start=True, stop=False)
                    nc.tensor.matmul(ptb[:, 128:256], y[:, 2:4], t_ident[:],
                                     is_transpose=True, start=False, stop=True)
                    mx = work.tile([2, 1], f32, name=f"mx{t}", tag="mx")
                    nc.vector.reduce_max(mx[:], ptb[:], AX.X)
                    eqB = work.tile([2, 256], f32, name=f"eqB{t}", tag="eqB")
                    nc.vector.tensor_scalar(eqB[:], ptb[:], mx[:], None, ALU.is_equal)
                    po = pso.tile([128, 4], f32, name=f"po{t}", tag="po")
                    nc.tensor.matmul(po[:, 0:2], eqB[:, 0:128], t_ident[0:2, 0:2],
                                     is_transpose=True, start=True, stop=False)
                    nc.tensor.matmul(po[:, 2:4], eqB[:, 128:256], t_ident[0:2, 0:2],
                                     is_transpose=True, start=False, stop=True)
                    nc.vector.tensor_copy(t_oneh[:], po[:])
                    pi = psi.tile([2, 1], f32, name=f"pi{t}", tag="pi")
                    nc.tensor.matmul(pi[:], t_oneh[:, 0:2], t_iota[:, 0:1], start=True, stop=False)
                    nc.tensor.matmul(pi[:], t_oneh[:, 2:4], t_iota[:, 1:2], start=False, stop=True)
                    nc.vector.tensor_copy(samplesw[:, t:t + 1], pi[:])
                # stream the window out
                nc.sync.dma_start(out_logits[:, ds(i * (U * 4), U * 4)], logitsw[:])
                nc.sync.dma_start(out_samples[:, ds(i * U, U)], samplesw[:])

            if n_iter > 1:
                with tc.For_i(0, n_iter, 1) as iv:
                    body(iv)
            else:
                body(0)

            # ---- outputs ----
            nc.sync.dma_start(out_h[:], t_h[:])

    nc.finalize()
    _CACHE[key] = nc
    return nc


def _pack_inputs(inputs, t_steps):
    """Build the per-core in_maps. Returns list of dicts."""
    f32 = np.float32
    cond = np.asarray(inputs["cond"], f32)
    h0 = np.asarray(inputs["h0"], f32)
    W_ih = np.asarray(inputs["W_ih"], f32)
    W_hh = np.asarray(inputs["W_hh"], f32)
    b_ih = np.asarray(inputs["b_ih"], f32)
    b_hh = np.asarray(inputs["b_hh"], f32)
    W_hid = np.asarray(inputs["W_hid"], f32)
    b_hid = np.asarray(inputs["b_hid"], f32)
    W_out = np.asarray(inputs["W_out"], f32)
    b_out = np.asarray(inputs["b_out"], f32)
    emb = np.asarray(inputs["embedding"], f32)

    E2 = (emb @ W_ih.T).astype(f32)  # [V, 3G]

    # weight packs (lhsT tiles laid side by side)
    def pack_tiles(mat_kl_m, nk, nm):
        # mat[k_global, m_global] -> [128, nk*nm*128] with tile (kc, mc)
        out = np.empty((128, nk * nm * 128), f32)
        i = 0
        for kc in range(nk):
            for mc in range(nm):
                out[:, i * 128:(i + 1) * 128] = mat_kl_m[kc * 128:(kc + 1) * 128, mc * 128:(mc + 1) * 128]
                i += 1
        return out

    wih_p = pack_tiles(W_ih.T.copy(), 1, OC3)          # [C, 3G] tiles (oc)
    wehh_p = pack_tiles(W_hh.T.copy(), GC, OC3)        # [G, 3G]
    e2_p = pack_tiles(E2, VC, OC3)                     # [V, 3G]
    whid_p = pack_tiles(W_hid.T.copy(), GC, HC)        # [G, H]
    wout_p = pack_tiles(W_out.T.copy(), HC, VC)        # [H, V]

    ident = np.eye(128, dtype=f32)
    iota = np.empty((128, VC), f32)
    for vc in range(VC):
        iota[:, vc] = np.arange(128, dtype=f32) + 128 * vc

    def bcast_pairs(vec, nch):  # [128, nch*2]: col c*2+b = vec[c*128+p]
        o = np.empty((128, nch * 2), f32)
        for c in range(nch):
            seg = vec[c * 128:(c + 1) * 128]
            o[:, c * 2] = seg
            o[:, c * 2 + 1] = seg
        return o

    brz_p = bcast_pairs((b_ih[:1024] + b_hh[:1024]).astype(f32), 8)
    bin_p = bcast_pairs(b_ih[1024:], GC)
    bhhn_p = bcast_pairs(b_hh[1024:], GC)
    bhid_p = bcast_pairs(b_hid, HC)
    bout_p = bcast_pairs(b_out, VC)

    gum_all = _gumbel_noise()  # [T, B, V]

    in_maps = []
    for core in range(NCORES):
        bsl = slice(core * BPC, (core + 1) * BPC)
        # cond rows r = t*2+b
        cond_c = np.zeros((TPAD * BPC, 128), f32)
        cc = cond[bsl]  # [2, T, C]
        cond_c[: t_steps * BPC] = np.transpose(cc[:, :t_steps], (1, 0, 2)).reshape(t_steps * BPC, C)
        # gumbel [128, t*4+vc*2+b]
        gg = gum_all[:t_steps, bsl]  # [t, 2, V]
        gum_c = np.transpose(gg.reshape(t_steps, BPC, VC, 128), (3, 0, 2, 1)).reshape(128, t_steps * 4).copy()
        # h0 feature-major
        h0_c = np.empty((128, 2 * GC), f32)
        for gc in range(GC):
            for b in range(BPC):
                h0_c[:, gc * 2 + b] = h0[core * BPC + b, gc * 128:(gc + 1) * 128]
        oneh0_c = np.zeros((128, 2 * VC), f32)
        prev0 = V // 2  # 128 -> vc=1, p=0
        oneh0_c[prev0 % 128, (prev0 // 128) * 2 + 0] = 1.0
        oneh0_c[prev0 % 128, (prev0 // 128) * 2 + 1] = 1.0
        in_maps.append({
            "wih": wih_p, "wehh": wehh_p, "e2m": e2_p, "whid": whid_p,
            "wout": wout_p, "ident": ident, "iota": iota, "brz": brz_p,
            "bin": bin_p, "bhhn": bhhn_p, "bhid": bhid_p, "bout": bout_p,
            "cond_n": cond_c, "gum": gum_c, "h0f": h0_c, "oneh0": oneh0_c,
        })
    return in_maps


def _unpack_outputs(results, t_steps):
    f32 = np.float32
    logits = np.empty((B, t_steps, V), f32)
    samples = np.empty((B, t_steps), np.int32)
    h_final = np.empty((B, G), f32)
    for core in range(NCORES):
        r = results[core]
        lo = r["out_logits"].reshape(128, t_steps, VC, BPC)  # [p, t, vc, b]
        for b in range(BPC):
            bg = core * BPC + b
            logits[bg] = np.transpose(lo[:, :, :, b], (1, 2, 0)).reshape(t_steps, V)
            samples[bg] = np.rint(r["out_samples"][b]).astype(np.int32)
            hf = r["out_h"].reshape(128, GC, BPC)
            h_final[bg] = np.transpose(hf[:, :, b], (1, 0)).reshape(G)
    return logits, samples, h_final


def _run(inputs, t_steps):
    from concourse.bass_utils import run_bass_kernel_spmd

    nc = _build_program(t_steps)
    in_maps = _pack_inputs(inputs, t_steps)
    res = run_bass_kernel_spmd(nc, in_maps, core_ids=list(range(NCORES)))
    return _unpack_outputs(res.results, t_steps)


def kernel(**inputs):
    return _run(inputs, T)


def timed_exec(inputs, n=4, t_steps=None):
    """Device-exec timing: compile once, pre-stage inputs on device, time reruns."""
    import time
    import jax
    from jax.sharding import NamedSharding
    import concourse.mybir as mybir
    from concourse import bass2jax as B2J

    t_steps = t_steps or T
    nc = _build_program(t_steps)
    in_maps = _pack_inputs(inputs, t_steps)
    B2J.install_neuronx_cc_hook()
    partition_name = nc.partition_id_tensor.name if nc.partition_id_tensor else None
    in_names, out_names, out_avals, zero_outs = [], [], [], []
    for alloc in nc.m.functions[0].allocations:
        if not isinstance(alloc, mybir.MemoryLocationSet):
            continue
        name = alloc.memorylocations[0].name
        if alloc.kind == "ExternalInput":
            if name != partition_name:
                in_names.append(name)
        elif alloc.kind == "ExternalOutput":
            out_names.append(name)
            shape = tuple(alloc.tensor_shape)
            dtype = mybir.dt.np(alloc.dtype)
            out_avals.append(jax.core.ShapedArray(shape, dtype))
            zero_outs.append(np.zeros(shape, dtype))
    n_params = len(in_names)
    n_outs = len(out_names)
    all_in = list(in_names) + list(out_names) + ([partition_name] if partition_name else [])
    donate = tuple(range(n_params, n_params + n_outs))

    def _body(*args):
        operands = list(args)
        if partition_name:
            operands.append(B2J.partition_id_tensor())
        outs = B2J._bass_exec_p.bind(
            *operands,
            out_avals=tuple(out_avals),
            in_names=tuple(all_in),
            out_names=tuple(out_names),
            lowering_input_output_aliases=(),
            sim_require_finite=True,
            sim_require_nnan=True,
            nc=nc,
        )
        return tuple(outs)

    devices = jax.devices()[:NCORES]
    mesh = B2J.Mesh(np.asarray(devices), ("core",))
    spec = B2J.PartitionSpec("core")
    sharded = jax.jit(
        B2J.shard_map(_body, mesh=mesh, in_specs=(spec,) * (n_params + n_outs),
                      out_specs=(spec,) * n_outs, check_rep=False),
        donate_argnums=donate, keep_unused=True,
    )
    shd = NamedSharding(mesh, spec)
    staged = [
        jax.device_put(
            np.concatenate([np.asarray(in_maps[c][nm]) for c in range(NCORES)], axis=0), shd
        )
        for nm in in_names
    ]
    jax.block_until_ready(staged)

    def make_zeros():
        zs = [
            jax.device_put(np.zeros((NCORES * z.shape[0], *z.shape[1:]), z.dtype), shd)
            for z in zero_outs
        ]
        jax.block_until_ready(zs)
        return zs

    # warmup (compiles)
    outs = sharded(*staged, *make_zeros())
    jax.block_until_ready(outs)
    best = None
    for _ in range(n):
        zs = make_zeros()
        t0 = time.perf_counter()
        outs = sharded(*staged, *zs)
        jax.block_until_ready(outs)
        dt = time.perf_counter() - t0
        best = dt if best is None else min(best, dt)
    return best * 1e9
